# revision 1
# baseline (speedup 1.0000x reference)
"""Trainium2 Bass kernel for nn_DAWN_10419590660472 (moe_routing transformer).

Sharding: 8 cores = 4 batches x 2 vocab-halves. Each core computes the full
4-layer body for its batch (the causal residual stream cannot be
sequence-split without communication), then the tied-embedding head for all
1024 tokens over its 16000-entry vocab half. All cores run the SAME program;
only input data differs. Heavy matmuls in bf16 with fp32 PSUM accumulation.

Host-folded math:
- attn.mean(-1) == 1/S exactly (softmax rows sum to 1), so the routing gate
  sigmoid(ctx) is a per-layer constant folded into the sem projection.
- LN affine params fold into downstream weights (identity here: s=1, b=0).
- top_k(fin, 8): nc.vector.max yields the 8 largest per token; token_recipe
  = masked_softmax(fin) @ rec_sm, a matmul (no gather).
- Attention softmax needs no max-subtraction (scores are O(1)); denominators
  come free from a ones-augmented value matrix in the AV matmul and are
  folded into the PSUM->SBUF copy as a reciprocal multiply.
"""

import numpy as np
import ml_dtypes

VOC = 32000; D = 512; DFF = 2048; L = 4; H = 8; DH = D // H
NN = 256; NB = 32; R = 128; B = 4; S = 1024
NCORES = 8
TT = S // 128          # 8 token tiles
DS = D // 128          # 4 d-slices
FS = DFF // 128        # 16 dff-slices
QC = S // 512          # 2 query chunks
NG = NB // 4           # 8 groups of 4 basis matrices
VH = VOC // 2          # vocab half per core
VCH = 500              # head vocab chunk (<=512)
NVC = VH // VCH        # 32
VG = 8                 # emb streaming groups
VPG = NVC // VG        # 4 chunks per group
EPS = 1e-5

BF16 = ml_dtypes.bfloat16
_cache = {}
DEBUG = False


def _softmax_np(x, axis=-1):
    m = x.max(axis=axis, keepdims=True)
    e = np.exp(x - m)
    return e / e.sum(axis=axis, keepdims=True)


def _preprocess(inputs):
    f32 = lambda k: np.asarray(inputs[k], dtype=np.float32)
    ids = np.asarray(inputs["input_ids"])
    token_emb = f32("token_emb"); pos_emb = f32("pos_emb")
    basis_A = f32("basis_A"); basis_emb = f32("basis_emb")
    q_w = f32("q_w"); k_w = f32("k_w"); ao_w = f32("ao_w")
    recipe = f32("recipe"); ctx_pat = f32("ctx_pat")
    vout_w = f32("vout_w"); up_w = f32("up_w"); down_w = f32("down_w")
    ln1_s = f32("ln1_s"); ln2_s = f32("ln2_s"); lnf_s = f32("lnf_s")

    for k in ("q_b", "k_b", "ao_b", "vout_b", "up_b", "down_b",
              "ln1_b", "ln2_b", "lnf_b"):
        assert not np.any(np.asarray(inputs[k])), f"nonzero {k} unsupported"

    scale = 1.0 / np.sqrt(DH)
    x0 = token_emb[ids] + pos_emb[:S][None]              # [B, S, D]

    def part_first(a, nslice):
        # [nslice*128, F] -> [128, nslice, F]
        return np.ascontiguousarray(
            a.reshape(nslice, 128, -1).transpose(1, 0, 2))

    wq = np.empty((L, 128, DS, D), dtype=BF16)
    wk = np.empty((L, 128, DS, D), dtype=BF16)
    wao = np.empty((L, 128, DS, D), dtype=BF16)
    gT = np.empty((L, 128, DS, NN), dtype=BF16)
    recT = np.empty((L, 128, 2, NB), dtype=BF16)
    a_cat = np.empty((L, 128, DS, NB * R), dtype=BF16)
    wvout = np.empty((L, 128, D), dtype=BF16)
    wup = np.empty((L, 128, DS, DFF), dtype=BF16)
    wdn = np.empty((L, 128, FS, D), dtype=BF16)

    for l in range(L):
        wq[l] = part_first((q_w[l] * ln1_s[l][None, :] * scale).T, DS)
        wk[l] = part_first((k_w[l] * ln1_s[l][None, :]).T, DS)
        wao[l] = part_first(ao_w[l].T, DS)
        rs = _softmax_np(recipe[l])                      # [NN, NB]
        emb_sem = rs @ basis_emb                         # [NN, D]
        gate = 1.0 / (1.0 + np.exp(-(ctx_pat[l].sum(-1) / S)))
        gT[l] = part_first(((emb_sem * ln1_s[l][None, :]) * gate[:, None]).T, DS)
        recT[l] = part_first(rs, 2)
        ae = basis_A * ln1_s[l][None, :, None]           # [NB, D, R]
        a_cat[l] = part_first(ae.transpose(1, 0, 2).reshape(D, NB * R), DS)
        wvout[l] = vout_w[l].T.astype(BF16)              # [R, D]
        wup[l] = part_first((up_w[l] * ln2_s[l][None, :]).T, DS)
        wdn[l] = part_first(down_w[l].T, FS)

    eT_full = part_first((token_emb * lnf_s[None, :]).T, DS).astype(BF16)
    ident = np.eye(128, dtype=BF16)

    shared = dict(wq=wq, wk=wk, wao=wao, gT=gT, recT=recT, a_cat=a_cat,
                  wvout=wvout, wup=wup, wdn=wdn, ident=ident)
    per_core = []
    for c in range(NCORES):
        b, half = c // 2, c % 2
        m = dict(shared)
        m["x0"] = np.ascontiguousarray(x0[b]).astype(np.float32)
        m["eT"] = np.ascontiguousarray(eT_full[:, :, half * VH:(half + 1) * VH])
        per_core.append(m)
    return per_core


def _build_nc():
    import concourse.mybir as mybir
    import concourse.tile as tile
    from concourse import bacc
    from concourse.alu_op_type import AluOpType as Alu

    AF = mybir.ActivationFunctionType
    bf = mybir.dt.bfloat16
    f32 = mybir.dt.float32

    nc = bacc.Bacc("TRN2", target_bir_lowering=False, debug=False,
                   num_devices=NCORES)

    din = lambda n, shp, dt=bf: nc.dram_tensor(n, shp, dt, kind="ExternalInput")
    dr = dict(
        x0=din("x0", [S, D], f32),
        wq=din("wq", [L, 128, DS, D]), wk=din("wk", [L, 128, DS, D]),
        wao=din("wao", [L, 128, DS, D]), gT=din("gT", [L, 128, DS, NN]),
        recT=din("recT", [L, 128, 2, NB]),
        a_cat=din("a_cat", [L, 128, DS, NB * R]),
        wvout=din("wvout", [L, 128, D]), wup=din("wup", [L, 128, DS, DFF]),
        wdn=din("wdn", [L, 128, FS, D]), eT=din("eT", [128, DS, VH]),
        ident=din("ident", [128, 128]),
        out=nc.dram_tensor("logits", [S, VH], f32, kind="ExternalOutput"),
    )
    if DEBUG:
        dout = lambda n, shp, dt=bf: nc.dram_tensor(n, shp, dt,
                                                    kind="ExternalOutput")
        dr["d_nrmT"] = dout("d_nrmT", [128, DS, S])
        dr["d_qT"] = dout("d_qT", [128, DS, S])
        dr["d_kT"] = dout("d_kT", [128, DS, S])
        dr["d_tr"] = dout("d_tr", [128, TT, NB], f32)
        dr["d_vsT"] = dout("d_vsT", [128, S])
        dr["d_vv"] = dout("d_vv", [128, TT, H * (DH + 1)])
        dr["d_aoT"] = dout("d_aoT", [128, DS, S])
        dr["d_eT"] = dout("d_eT", [128, TT, 512])
        dr["d_x1"] = dout("d_x1", [128, TT, D], f32)
        dr["d_fin"] = dout("d_fin", [128, NN], f32)
        dr["d_psa"] = dout("d_psa", [65, 512], f32)
        dr["d_rb"] = dout("d_rb", [64, 512], f32)
        dr["d_m8"] = dout("d_m8", [128, 8], f32)
        dr["d_wf"] = dout("d_wf", [128, NN])

    with tile.TileContext(nc) as tc:
        _emit(nc, tc, mybir, Alu, AF, bf, f32, dr)

    nc.compile()
    return nc


def _emit(nc, tc, mybir, Alu, AF, bf, f32, dr):
    from contextlib import ExitStack
    ctx = ExitStack()
    pool = lambda name, bufs, space="SBUF": ctx.enter_context(
        tc.tile_pool(name=name, bufs=bufs, space=space))

    P_x = pool("x", 1)
    P_const = pool("const", 1)
    P_w = pool("w", 1)                 # small per-layer weights
    P_big = pool("big", 2)             # a_cat halves / wup / wdn / emb chunks
    P_act = pool("act", 1)             # per-layer activations
    P_nrm = pool("nrm", 2)             # token-major LN outputs (transient)
    P_attn = pool("attn", 2)           # eT buffers
    P_rt = pool("rt", 2)               # routing temporaries
    P_sm = pool("sm", 2)               # small stats tiles
    P_hd = pool("hd", 3)               # head staging
    P_ps = pool("ps", 2, "PSUM")       # generic matmul psum
    P_psT = pool("psT", 2, "PSUM")     # transpose psum
    P_psA = pool("psA", 2, "PSUM")     # xA psum
    P_psS = pool("psS", 2, "PSUM")     # scores / AV psum

    ident = P_const.tile([128, 128], bf)
    nc.sync.dma_start(out=ident, in_=dr["ident"][:, :])
    eps_sb = P_const.tile([128, 1], f32)
    nc.vector.memset(eps_sb, EPS)

    x_sb = P_x.tile([128, TT, D], f32)
    for t in range(TT):
        nc.sync.dma_start(out=x_sb[:, t, :],
                          in_=dr["x0"][t * 128:(t + 1) * 128, :])

    def layernorm(src_ap, dst_bf):
        stats = P_sm.tile([128, 6], f32, tag="st")
        nc.vector.bn_stats(out=stats, in_=src_ap)
        mv = P_sm.tile([128, 2], f32, tag="mv")
        nc.vector.bn_aggr(out=mv, in_=stats)
        rstd = P_sm.tile([128, 1], f32, tag="rs")
        nc.scalar.activation(out=rstd, in_=mv[:, 1:2], func=AF.Sqrt,
                             bias=eps_sb)
        nc.vector.reciprocal(out=rstd, in_=rstd)
        nc.vector.tensor_scalar(out=dst_bf, in0=src_ap, scalar1=mv[:, 0:1],
                                scalar2=rstd, op0=Alu.subtract, op1=Alu.mult)

    def transpose128(dst_sb, src_sb):
        ps = P_psT.tile([128, 128], bf)
        nc.tensor.transpose(ps, src_sb, ident)
        nc.scalar.copy(out=dst_sb, in_=ps)

    for l in range(L):
        wq_l = P_w.tile([128, DS, D], bf, tag="wq")
        wk_l = P_w.tile([128, DS, D], bf, tag="wk")
        wao_l = P_w.tile([128, DS, D], bf, tag="wao")
        g_l = P_w.tile([128, DS, NN], bf, tag="g")
        rec_l = P_w.tile([128, 2, NB], bf, tag="rec")
        wv_l = P_w.tile([128, D], bf, tag="wv")
        nc.sync.dma_start(out=wq_l, in_=dr["wq"][l])
        nc.sync.dma_start(out=wk_l, in_=dr["wk"][l])
        nc.sync.dma_start(out=wao_l, in_=dr["wao"][l])
        nc.sync.dma_start(out=g_l, in_=dr["gT"][l])
        nc.sync.dma_start(out=rec_l, in_=dr["recT"][l])
        nc.sync.dma_start(out=wv_l, in_=dr["wvout"][l])

        nrmT = P_act.tile([128, DS, S], bf, tag="nrmT")
        qT = P_act.tile([128, DS, S], bf, tag="qT")  # slot reused by FFN hT
        kT = P_act.tile([128, DS, S], bf, tag="kT")
        vv = P_act.tile([128, TT, H * (DH + 1)], bf, tag="vv")
        aoT = P_act.tile([128, DS, S], bf, tag="aoT")
        vsT = P_act.tile([128, S], bf, tag="vsT")
        tr_all = P_act.tile([128, TT, NB], f32, tag="tr")

        # ---- LN1 + transpose to [d, tok] ----
        for t in range(TT):
            nrm = P_nrm.tile([128, D], bf, tag="nrm")
            layernorm(x_sb[:, t, :], nrm)
            for ds in range(DS):
                transpose128(nrmT[:, ds, t * 128:(t + 1) * 128],
                             nrm[:, ds * 128:(ds + 1) * 128])

        # ---- Q/K projections (outputs stay [d_out, tok]) ----
        for qc in range(QC):
            for ot in range(DS):
                for (w_l, dstT) in ((wq_l, qT), (wk_l, kT)):
                    ps = P_ps.tile([128, 512], f32, tag="ps")
                    for ds in range(DS):
                        nc.tensor.matmul(
                            ps, w_l[:, ds, ot * 128:(ot + 1) * 128],
                            nrmT[:, ds, qc * 512:(qc + 1) * 512],
                            start=(ds == 0), stop=(ds == DS - 1))
                    nc.scalar.copy(out=dstT[:, ot, qc * 512:(qc + 1) * 512],
                                   in_=ps)

        # ---- routing: fin -> top8 -> masked softmax -> token_recipe ----
        for t in range(TT):
            fin_ps = P_ps.tile([128, 512], f32, tag="ps")
            for ds in range(DS):
                nc.tensor.matmul(fin_ps[:, :NN],
                                 nrmT[:, ds, t * 128:(t + 1) * 128],
                                 g_l[:, ds, :],
                                 start=(ds == 0), stop=(ds == DS - 1))
            fin = P_rt.tile([128, NN], f32, tag="fin")
            nc.vector.tensor_copy(out=fin, in_=fin_ps[:, :NN])
            m8 = P_rt.tile([128, 8], f32, tag="m8")
            nc.vector.max(out=m8, in_=fin)
            t8 = P_sm.tile([128, 1], f32, tag="t8")
            nc.vector.reduce_sum(out=t8, in_=m8, axis=mybir.AxisListType.X,
                                 op=Alu.min)   # 8th largest, order-agnostic
            nt8 = P_sm.tile([128, 1], f32, tag="nt8")
            nc.vector.tensor_scalar_mul(out=nt8, in0=t8, scalar1=-1.0)
            er = P_rt.tile([128, NN], f32, tag="er")
            nc.scalar.activation(out=er, in_=fin, func=AF.Exp, bias=nt8)
            we = P_rt.tile([128, NN], f32, tag="we")
            nc.vector.scalar_tensor_tensor(out=we, in0=fin, scalar=t8,
                                           in1=er, op0=Alu.is_ge, op1=Alu.mult)
            dn = P_sm.tile([128, 1], f32, tag="dn")
            nc.vector.reduce_sum(out=dn, in_=we, axis=mybir.AxisListType.X)
            rc = P_sm.tile([128, 1], f32, tag="rc")
            nc.vector.reciprocal(out=rc, in_=dn)
            wfull = P_rt.tile([128, NN], bf, tag="wfull")
            nc.vector.tensor_scalar_mul(out=wfull, in0=we, scalar1=rc)
            if DEBUG and l == 0 and t == 0:
                nc.sync.dma_start(out=dr["d_fin"][:], in_=fin)
                nc.sync.dma_start(out=dr["d_m8"][:], in_=m8)
                nc.sync.dma_start(out=dr["d_wf"][:], in_=wfull)
            wfT = P_rt.tile([128, 2, 128], bf, tag="wfT")
            for ns in range(2):
                transpose128(wfT[:, ns, :], wfull[:, ns * 128:(ns + 1) * 128])
            tr_ps = P_ps.tile([128, 512], f32, tag="ps")
            for ns in range(2):
                nc.tensor.matmul(tr_ps[:, :NB], wfT[:, ns, :], rec_l[:, ns, :],
                                 start=(ns == 0), stop=(ns == 1))
            nc.vector.tensor_copy(out=tr_all[:, t, :], in_=tr_ps[:, :NB])

        # ---- xA (4 basis mats per matmul; A streamed in halves) ----
        vs_all = P_act.tile([128, TT, R], f32, tag="vsall")
        for ah in range(2):
            a_l = P_big.tile([128, DS, NB * R // 2], bf, tag="big")
            nc.sync.dma_start(
                out=a_l,
                in_=dr["a_cat"][l][:, :, ah * (NB * R // 2):(ah + 1) * (NB * R // 2)])
            for t in range(TT):
                for g in range(NG // 2):
                    psA = P_psA.tile([128, 512], f32, tag="psA")
                    for ds in range(DS):
                        nc.tensor.matmul(psA,
                                         nrmT[:, ds, t * 128:(t + 1) * 128],
                                         a_l[:, ds, g * 512:(g + 1) * 512],
                                         start=(ds == 0), stop=(ds == DS - 1))
                    for ni in range(4):
                        n = ah * 16 + g * 4 + ni
                        if n == 0:
                            nc.vector.tensor_scalar_mul(
                                out=vs_all[:, t, :], in0=psA[:, :R],
                                scalar1=tr_all[:, t, 0:1])
                        else:
                            nc.vector.scalar_tensor_tensor(
                                out=vs_all[:, t, :],
                                in0=psA[:, ni * R:(ni + 1) * R],
                                scalar=tr_all[:, t, n:n + 1],
                                in1=vs_all[:, t, :],
                                op0=Alu.mult, op1=Alu.add)
        # ---- v_sem transpose + Vv ----
        for t in range(TT):
            vs_bf = P_rt.tile([128, R], bf, tag="vsbf")
            nc.vector.tensor_copy(out=vs_bf, in_=vs_all[:, t, :])
            transpose128(vsT[:, t * 128:(t + 1) * 128], vs_bf)
            psv = P_ps.tile([128, 512], f32, tag="ps")
            nc.tensor.matmul(psv, vsT[:, t * 128:(t + 1) * 128], wv_l,
                             start=True, stop=True)
            # per-head layout [Vv_h | 1]: the ones column makes the AV matmul
            # also produce the softmax denominator (psum partition 64)
            vvh = vv[:, t, :].rearrange("p (h e) -> p h e", h=H)
            nc.scalar.copy(out=vvh[:, :, 0:DH],
                           in_=psv.rearrange("p (h e) -> p h e", h=H))
            nc.vector.memset(vvh[:, :, DH:DH + 1], 1.0)

        # ---- attention (scores transposed; denom via ones-augmented AV) ----
        for qc in range(QC):
            nkt = qc * 4 + 4
            for h in range(H):
                hp = (h % 2) * 64
                hd = h // 2
                eT = P_attn.tile([128, TT, 512], bf, tag="eT")
                dump_eT = DEBUG and l == 0 and qc == 1 and h == 0
                for kt in range(nkt):
                    pss = P_psS.tile([128, 512], f32, tag="psS")
                    nc.tensor.matmul(
                        pss, kT[hp:hp + 64, hd, kt * 128:(kt + 1) * 128],
                        qT[hp:hp + 64, hd, qc * 512:(qc + 1) * 512],
                        start=True, stop=True)
                    nc.scalar.activation(out=eT[:, kt, :], in_=pss, func=AF.Exp)
                    kt_rel = kt - qc * 4
                    if kt_rel >= 0:
                        if kt_rel > 0:
                            nc.vector.memset(eT[:, kt, 0:kt_rel * 128], 0.0)
                        nc.gpsimd.affine_select(
                            out=eT[:, kt, kt_rel * 128:(kt_rel + 1) * 128],
                            in_=eT[:, kt, kt_rel * 128:(kt_rel + 1) * 128],
                            compare_op=Alu.is_ge, fill=0.0, base=0,
                            pattern=[[1, 128]], channel_multiplier=-1)
                if dump_eT:
                    nc.sync.dma_start(out=dr["d_eT"][:], in_=eT)
                psa = P_psS.tile([128, 512], f32, tag="psS")
                for kt in range(nkt):
                    nc.tensor.matmul(
                        psa[0:DH + 1, :],
                        vv[:, kt, h * (DH + 1):(h + 1) * (DH + 1)],
                        eT[:, kt, :], start=(kt == 0), stop=(kt == nkt - 1))
                # copy out of PSUM early (frees the bank), then:
                # denom row -> partition 0 via DMA -> reciprocal -> broadcast
                aoU = P_sm.tile([65, 512], f32, tag="aoU")
                nc.scalar.copy(out=aoU, in_=psa[0:DH + 1, :])
                dn0 = P_sm.tile([1, 512], f32, tag="dn0")
                nc.gpsimd.dma_start(out=dn0, in_=aoU[DH:DH + 1, :])
                rcq = P_sm.tile([1, 512], f32, tag="rcq")
                nc.vector.reciprocal(out=rcq, in_=dn0)
                rb = P_sm.tile([64, 512], f32, tag="rb")
                nc.gpsimd.partition_broadcast(rb, rcq)
                if dump_eT:
                    nc.sync.dma_start(out=dr["d_psa"][:], in_=aoU)
                    nc.sync.dma_start(out=dr["d_rb"][:], in_=rb)
                if hp == 0:
                    nc.vector.tensor_tensor(
                        out=aoT[0:64, hd, qc * 512:(qc + 1) * 512],
                        in0=aoU[0:DH, :], in1=rb, op=Alu.mult)
                else:
                    tmp = P_sm.tile([64, 512], bf, tag="aotmp")
                    nc.vector.tensor_tensor(out=tmp, in0=aoU[0:DH, :],
                                            in1=rb, op=Alu.mult)
                    nc.sync.dma_start(
                        out=aoT[64:128, hd, qc * 512:(qc + 1) * 512], in_=tmp)

        if DEBUG and l == 0:
            nc.sync.dma_start(out=dr["d_nrmT"][:], in_=nrmT)
            nc.sync.dma_start(out=dr["d_qT"][:], in_=qT)
            nc.sync.dma_start(out=dr["d_kT"][:], in_=kT)
            nc.sync.dma_start(out=dr["d_tr"][:], in_=tr_all)
            nc.sync.dma_start(out=dr["d_vsT"][:], in_=vsT)
            nc.sync.dma_start(out=dr["d_vv"][:], in_=vv)
            nc.sync.dma_start(out=dr["d_aoT"][:], in_=aoT)

        # ---- attention out projection + residual ----
        for t in range(TT):
            pso = P_ps.tile([128, 512], f32, tag="ps")
            for ds in range(DS):
                nc.tensor.matmul(pso, aoT[:, ds, t * 128:(t + 1) * 128],
                                 wao_l[:, ds, :],
                                 start=(ds == 0), stop=(ds == DS - 1))
            nc.vector.tensor_tensor(out=x_sb[:, t, :], in0=pso,
                                    in1=x_sb[:, t, :], op=Alu.add)

        # ---- FFN ----
        wup_l = P_big.tile([128, DS, DFF], bf, tag="big")
        wdn_l = P_big.tile([128, FS, D], bf, tag="big")
        nc.sync.dma_start(out=wup_l, in_=dr["wup"][l])
        nc.sync.dma_start(out=wdn_l, in_=dr["wdn"][l])
        n2T = P_act.tile([128, DS, S], bf, tag="n2T")
        for t in range(TT):
            nrm = P_nrm.tile([128, D], bf, tag="nrm")
            layernorm(x_sb[:, t, :], nrm)
            for ds in range(DS):
                transpose128(n2T[:, ds, t * 128:(t + 1) * 128],
                             nrm[:, ds * 128:(ds + 1) * 128])
        for qc in range(QC):
            hT = P_act.tile([128, FS, 512], bf, tag="qT")
            for ft in range(FS):
                psu = P_ps.tile([128, 512], f32, tag="ps")
                for ds in range(DS):
                    nc.tensor.matmul(psu,
                                     wup_l[:, ds, ft * 128:(ft + 1) * 128],
                                     n2T[:, ds, qc * 512:(qc + 1) * 512],
                                     start=(ds == 0), stop=(ds == DS - 1))
                nc.scalar.activation(out=hT[:, ft, :], in_=psu, func=AF.Gelu)
            for tr in range(4):
                t = qc * 4 + tr
                psd = P_ps.tile([128, 512], f32, tag="ps")
                for fs in range(FS):
                    nc.tensor.matmul(psd, hT[:, fs, tr * 128:(tr + 1) * 128],
                                     wdn_l[:, fs, :],
                                     start=(fs == 0), stop=(fs == FS - 1))
                nc.vector.tensor_tensor(out=x_sb[:, t, :], in0=psd,
                                        in1=x_sb[:, t, :], op=Alu.add)
        if DEBUG and l == 0:
            nc.sync.dma_start(out=dr["d_x1"][:], in_=x_sb)

    # ---- final LN + tied head over this core's vocab half ----
    xfT = P_act.tile([128, DS, S], bf, tag="nrmT")
    for t in range(TT):
        nrm = P_nrm.tile([128, D], bf, tag="nrm")
        layernorm(x_sb[:, t, :], nrm)
        for ds in range(DS):
            transpose128(xfT[:, ds, t * 128:(t + 1) * 128],
                         nrm[:, ds * 128:(ds + 1) * 128])
    for vg in range(VG):
        emb = P_big.tile([128, DS, VG * VPG * VCH // VG], bf, tag="big")
        nc.sync.dma_start(
            out=emb, in_=dr["eT"][:, :, vg * VPG * VCH:(vg + 1) * VPG * VCH])
        for t in range(TT):
            for vi in range(VPG):
                psh = P_ps.tile([128, 512], f32, tag="ps")
                for ds in range(DS):
                    nc.tensor.matmul(
                        psh[:, :VCH], xfT[:, ds, t * 128:(t + 1) * 128],
                        emb[:, ds, vi * VCH:(vi + 1) * VCH],
                        start=(ds == 0), stop=(ds == DS - 1))
                stage = P_hd.tile([128, VCH], f32, tag="stage")
                if vi % 2 == 0:
                    nc.vector.tensor_copy(out=stage, in_=psh[:, :VCH])
                else:
                    nc.scalar.copy(out=stage, in_=psh[:, :VCH])
                off = (vg * VPG + vi) * VCH
                nc.sync.dma_start(
                    out=dr["out"][t * 128:(t + 1) * 128, off:off + VCH],
                    in_=stage)
    ctx.close()


def kernel(**inputs):
    from concourse.bass_utils import run_bass_kernel_spmd

    if "nc" not in _cache:
        _cache["nc"] = _build_nc()
    nc = _cache["nc"]

    in_maps = _preprocess(inputs)
    res = run_bass_kernel_spmd(nc, in_maps, core_ids=list(range(NCORES)))
    global _last_results
    _last_results = res.results

    out = np.empty((B, S, VOC), dtype=np.float32)
    for c in range(NCORES):
        b, half = c // 2, c % 2
        out[b, :, half * VH:(half + 1) * VH] = res.results[c]["logits"]
    return out

